# revision 16
# baseline (speedup 1.0000x reference)
"""Causal self-attention (B=4, T=2048, C=1024, H=16) on 8 NeuronCores.

Sharding: core = (batch b, head-group g): data-parallel over B=4, tensor-
parallel over heads (2 groups x 8 heads).  Each core computes QKV + attention
for its 8 heads and the matching half of the c_proj contraction; the host
sums the two partial c_proj outputs per batch and adds b_proj.

Device layout notes:
  - all matmul operands bf16 (PE runs fp32 at 1/4 rate), PSUM f32
  - x, weights are pre-transposed on the host so every matmul contraction
    sits on the partition dim; no on-device transposes anywhere
  - QKV biases enter as K=1 rank-1 matmuls against a ones row
  - S is computed transposed ([keys, queries]); exp(S/8) on ScalarE with no
    max-subtraction (logits bounded ~+-4 for this problem's scale)
  - causality at tile granularity: k-tiles above the diagonal are skipped,
    diagonal tiles multiplied by precomputed 0/1 masks after exp; diagonal
    tiles further restrict S/exp/PV to their valid column range
  - softmax denominator = ones column appended to each head's V; PV matmul
    emits [y.T | denom] per (head, q-chunk); normalization = batched DVE
    reciprocal + one-hot-selector broadcast matmul + elementwise multiply

Scheduling notes (the performance-critical part):
  - the attention inner loop is ScalarE(exp)-bound; the PE would idle ~35%
    and the HAM activity monitor then clock-throttles it to 1.2 GHz.  To
    keep the PE dense, later head-pairs' QKV projection matmuls (and the
    second half of V) are drip-fed one-per-iteration into the attention
    k-loop as "filler" work, and each head's normalization is deferred into
    the next head's filler stream (its reciprocal would otherwise stall the
    in-order PE queue >3.4us and re-trip the throttle)
  - S+exp run DEPTH=3 k-tiles ahead of the PV consumer (software pipeline)
  - emission order == dependency-tracking order, so ensure() force-emits a
    pair's projection fillers before that pair's attention reads them
"""

import os

import numpy as np
import ml_dtypes

B, T, C, H = 4, 2048, 1024, 16
D = 64          # head dim
HL = 8          # heads per core
CL = HL * D     # 512 local channels
TQ = 512        # query chunk (matmul moving dim)
TK = 128        # key tile (psum partition dim)
NQC = T // TQ   # 4 query chunks
NKT = T // TK   # 16 key tiles
VW = HL * (D + 1)  # 520: V with per-head ones column

_prog = None
last_results = None  # BassKernelResults of the most recent run (for test.py)


def _build_program():
    import concourse.mybir as mybir
    import concourse.tile as tile
    from concourse import bacc

    f32 = mybir.dt.float32
    bf16 = mybir.dt.bfloat16
    EXP = mybir.ActivationFunctionType.Exp

    nc = bacc.Bacc("TRN2", target_bir_lowering=False, debug=False)

    xt_d = nc.dram_tensor("xt", [8, 128, T], bf16, kind="ExternalInput")
    wqk_d = nc.dram_tensor("wqk", [8, 128, 2 * CL], bf16, kind="ExternalInput")
    wv_d = nc.dram_tensor("wv", [8, 128, VW], bf16, kind="ExternalInput")
    # per-partition bias columns: bqkc[:, dst*4+g] = bias for (q|k, pair g);
    # bvb = bv broadcast down 128 partitions (incl. the ones column) so the
    # bias adds ride the psum->sbuf copies instead of K=1 rank-1 matmuls
    bqkc_d = nc.dram_tensor("bqkc", [128, 8], f32, kind="ExternalInput")
    bvb_d = nc.dram_tensor("bvb", [128, VW], f32, kind="ExternalInput")
    wp_d = nc.dram_tensor("wp", [4, 128, C], bf16, kind="ExternalInput")
    mask_d = nc.dram_tensor("mask", [4, 128, TQ], bf16, kind="ExternalInput")
    out_d = nc.dram_tensor("out", [T, C], bf16, kind="ExternalOutput")

    with tile.TileContext(nc) as tc:
        with (
            tc.tile_pool(name="persist", bufs=1) as pp,
            tc.tile_pool(name="ptpool", bufs=10) as ptp,
            tc.tile_pool(name="stage", bufs=3) as sp,
            tc.tile_pool(name="small", bufs=3) as smp,
            tc.tile_pool(name="psA", bufs=4, space="PSUM") as psA,
            tc.tile_pool(name="psF", bufs=2, space="PSUM") as psF,
            tc.tile_pool(name="psY", bufs=2, space="PSUM") as psY,
        ):
            # ---- load everything ----
            xt = [pp.tile([128, T], bf16, name=f"xt{k}") for k in range(8)]
            wqk = [pp.tile([128, 2 * CL], bf16, name=f"wqk{k}") for k in range(8)]
            wv = [pp.tile([128, VW], bf16, name=f"wv{k}") for k in range(8)]
            wp = [pp.tile([128, C], bf16, name=f"wp{k}") for k in range(4)]
            maskt = [pp.tile([128, TQ], bf16, name=f"mask{j}") for j in range(4)]
            bqkc = pp.tile([128, 8], f32, name="bqkc")
            bvb = pp.tile([128, VW], f32, name="bvb")
            # ones row for the K=1 denominator-broadcast matmul
            ones64 = pp.tile([1, 64], bf16, name="ones64")

            # interleave wqk/xt so QKV matmul k-step j can start as soon as
            # its two tiles land (~2us) instead of after the full input load
            for k in range(8):
                nc.sync.dma_start(out=wqk[k][:], in_=wqk_d[k])
                nc.sync.dma_start(out=xt[k][:], in_=xt_d[k])
                if k == 0:
                    nc.sync.dma_start(out=bqkc[:], in_=bqkc_d[:])
                    nc.sync.dma_start(out=bvb[:], in_=bvb_d[:])
            for k in range(8):
                nc.sync.dma_start(out=wv[k][:], in_=wv_d[k])
            for j in range(4):
                nc.sync.dma_start(out=maskt[j][:], in_=mask_d[j])
            for k in range(4):
                nc.sync.dma_start(out=wp[k][:], in_=wp_d[k])
            nc.vector.memset(ones64[:], 1.0)

            # ---- QKV projection ----
            # QT/KT in [channel, t] layout; channel tile g = head pair g
            qt = [pp.tile([128, T], bf16, name=f"qt{i}") for i in range(4)]
            kt = [pp.tile([128, T], bf16, name=f"kt{i}") for i in range(4)]
            # V in natural [t, channel] layout with a ones column per head
            vsb = [pp.tile([128, VW], bf16, name=f"v{i}") for i in range(NKT)]
            yt = [pp.tile([128, T], bf16, name=f"yt{i}") for i in range(4)]

            def emit_qk_steps(g):
                """One head-pair's Q.T and K.T projection as a list of
                single-matmul closures (PE filler units)."""
                steps = []
                for dst, off, bi in ((qt, 0, 0), (kt, CL, 4)):
                    for j in range(NQC):
                        ph = {}

                        def step(k, ph=ph, dst=dst, off=off, j=j, g=g, bi=bi):
                            if k == 0:
                                ph["ps"] = psF.tile(
                                    [128, TQ], f32, name="ps_f", tag="fill"
                                )
                            nc.tensor.matmul(
                                ph["ps"][:],
                                lhsT=wqk[k][:, off + g * 128 : off + (g + 1) * 128],
                                rhs=xt[k][:, j * TQ : (j + 1) * TQ],
                                start=(k == 0),
                                stop=(k == 7),
                            )
                            if k == 7:
                                # bias add rides the psum->sbuf copy (DVE)
                                nc.vector.tensor_scalar_add(
                                    out=dst[g][:, j * TQ : (j + 1) * TQ],
                                    in0=ph["ps"][:],
                                    scalar1=bqkc[:, bi + g : bi + g + 1],
                                )

                        for k in range(8):
                            steps.append(lambda k=k, step=step: step(k))
                return steps

            def emit_v_steps(h2):
                """V projection for 4 heads as single-matmul closures."""
                w0 = h2 * (VW // 2)
                steps = []
                for it in range(NKT):
                    ph = {}

                    def step(k, ph=ph, it=it, w0=w0):
                        if k == 0:
                            ph["ps"] = psF.tile([128, TQ], f32, name="ps_v",
                                                tag="fill")
                        nc.tensor.matmul(
                            ph["ps"][:, : VW // 2],
                            lhsT=xt[k][:, it * 128 : (it + 1) * 128],
                            rhs=wv[k][:, w0 : w0 + VW // 2],
                            start=(k == 0),
                            stop=(k == 7),
                        )
                        if k == 7:
                            # bias (incl. the denominator ones column) rides
                            # the psum->sbuf copy via the broadcast tile
                            nc.vector.tensor_add(
                                out=vsb[it][:, w0 : w0 + VW // 2],
                                in0=ph["ps"][:, : VW // 2],
                                in1=bvb[:, w0 : w0 + VW // 2],
                            )

                    for k in range(8):
                        steps.append(lambda k=k, step=step: step(k))
                return steps

            # pre-phase (dense PE work, warms HAM): pair 0 QK + first V half
            # (heads 0-3); the second half drains via the filler stream well
            # before attn pair 2 (heads 4-7) consumes it
            for step in emit_qk_steps(0):
                step()
            for step in emit_v_steps(0):
                step()

            # ---- attention ----
            # the attention inner loop is ScalarE(exp)-bound; drip-feed the
            # NEXT pair's QKV matmuls into the PE queue as filler so the PE
            # stays dense (otherwise HAM clock-throttles it to 1.2 GHz)
            fillers = []  # list of (tag, closure)

            def drain(n):
                for _ in range(n):
                    if fillers:
                        fillers.pop(0)[1]()

            def ensure(tag):
                """Force-emit every queued step up to the last one of `tag`
                (emission order == dependency-tracking order, so a pair's
                projection steps MUST be emitted before its attention reads)."""
                while any(t == tag for t, _ in fillers):
                    fillers.pop(0)[1]()

            def norm_steps(h_, qc, ysb):
                """Per-(head, q-chunk) normalization as filler steps: copy the
                ysb denominator row down to partition 0 (custom DVE ops ignore
                the AP base partition on HW!), 1/den via approx_fast (~51 ULP,
                ~5x cheaper than exact), bf16 cast, K=1 ones-broadcast matmul
                to spread it over 64 partitions, then the elementwise multiply."""
                g2_, po_ = h_ // 2, 64 * (h_ % 2)
                den0 = smp.tile([1, TQ], f32, name="den0", tag="den0", bufs=3)
                rec1f = smp.tile([1, TQ], f32, name="rec1f", tag="recf", bufs=3)
                rec1b = smp.tile([1, TQ], bf16, name="rec1b", tag="recb", bufs=3)

                def bcast_mul():
                    bc = psA.tile([64, TQ], f32, name="bc", tag="mm512")
                    nc.tensor.matmul(
                        bc[:],
                        lhsT=ones64[0:1, :],
                        rhs=rec1b[0:1, :],
                        start=True,
                        stop=True,
                    )
                    nc.vector.tensor_mul(
                        yt[g2_][po_ : po_ + 64, qc * TQ : (qc + 1) * TQ],
                        ysb[0:64, :],
                        bc[:],
                    )

                return [
                    lambda: nc.vector.tensor_copy(out=den0[:], in_=ysb[64:65, :]),
                    lambda: nc.vector.reciprocal_approx_fast(
                        out=rec1f[:], in_=den0[:]
                    ),
                    lambda: nc.vector.tensor_copy(out=rec1b[:], in_=rec1f[:]),
                    bcast_mul,
                ]

            # head PAIRS: the two heads of a pair have their K/Q channel
            # halves at partitions 0:64 / 64:128 of the same kt/qt tile, so
            # their K=64 S matmuls go to PE row groups (0,0) / (64,0) and run
            # CONCURRENTLY in the array (row tiling) — the pair's two S tiles
            # cost ~one 512-column stream instead of two.
            DEPTH = 3
            for p in range(4):
                ha, hb = 2 * p, 2 * p + 1
                if p < 3:
                    fillers.extend((f"qk{p + 1}", s) for s in emit_qk_steps(p + 1))
                    if p == 0:
                        fillers.extend(("vh1", s) for s in emit_v_steps(1))
                ensure(f"qk{p}")
                if p >= 2:
                    ensure("vh1")
                for qc in range(NQC):
                    ktop = (qc + 1) * (TQ // TK)  # causal: k tiles 0..ktop-1
                    yps_a = psY.tile([D + 1, TQ], f32, name="yps_a", tag="y")
                    yps_b = psY.tile([D + 1, TQ], f32, name="yps_b", tag="y")
                    pts = {}

                    def s_stage(ktl, qc=qc, p=p, pts=pts):
                        j = ktl - qc * (TQ // TK)
                        # diagonal tiles have no valid columns before col0
                        col0 = j * TK if j >= 0 else 0
                        ps_a = psA.tile([128, TQ], f32, name="ps_a", tag="mm512")
                        nc.tensor.matmul(
                            ps_a[:, col0:],
                            lhsT=kt[p][0:64, ktl * TK : (ktl + 1) * TK],
                            rhs=qt[p][0:64, qc * TQ + col0 : (qc + 1) * TQ],
                            start=True,
                            stop=True,
                        )
                        ps_b = psA.tile([128, TQ], f32, name="ps_b", tag="mm512")
                        nc.tensor.matmul(
                            ps_b[:, col0:],
                            lhsT=kt[p][64:128, ktl * TK : (ktl + 1) * TK],
                            rhs=qt[p][64:128, qc * TQ + col0 : (qc + 1) * TQ],
                            start=True,
                            stop=True,
                        )
                        pt_a = ptp.tile([128, TQ], bf16, name="pt_a")
                        pt_b = ptp.tile([128, TQ], bf16, name="pt_b")
                        # P.T = exp(S.T/sqrt(D)); logits bounded, no max pass
                        nc.scalar.activation(
                            pt_a[:, col0:], ps_a[:, col0:], EXP, scale=0.125
                        )
                        nc.scalar.activation(
                            pt_b[:, col0:], ps_b[:, col0:], EXP, scale=0.125
                        )
                        if j >= 0:  # diagonal: zero the acausal corner
                            # on GPSIMD (idle): keeps the mask off the busy
                            # in-order DVE queue so PV never waits behind
                            # ysb/filler copies
                            nc.gpsimd.tensor_mul(
                                pt_a[:, col0:], pt_a[:, col0:], maskt[j][:, col0:]
                            )
                            nc.gpsimd.tensor_mul(
                                pt_b[:, col0:], pt_b[:, col0:], maskt[j][:, col0:]
                            )
                        pts[ktl] = (pt_a, pt_b, col0)

                    def pv_stage(ktl, ha=ha, hb=hb, pts=pts, yps_a=yps_a,
                                 yps_b=yps_b, ktop=ktop):
                        pt_a, pt_b, col0 = pts.pop(ktl)
                        nc.tensor.matmul(
                            yps_a[:, col0:],
                            lhsT=vsb[ktl][:, ha * 65 : (ha + 1) * 65],
                            rhs=pt_a[:, col0:],
                            start=(ktl == 0),
                            stop=(ktl == ktop - 1),
                        )
                        nc.tensor.matmul(
                            yps_b[:, col0:],
                            lhsT=vsb[ktl][:, hb * 65 : (hb + 1) * 65],
                            rhs=pt_b[:, col0:],
                            start=(ktl == 0),
                            stop=(ktl == ktop - 1),
                        )

                    for ktl in range(ktop):
                        s_stage(ktl)
                        drain(3 if len(fillers) >= 60 else 2)
                        if ktl >= DEPTH:
                            pv_stage(ktl - DEPTH)
                    for ktl in range(max(0, ktop - DEPTH), ktop):
                        pv_stage(ktl)
                    # rows 0..63 = unnormalized y.T, row 64 = denominator
                    ysb_a = smp.tile([D + 1, TQ], f32, name="ysb_a", tag="ysb",
                                     bufs=6)
                    ysb_b = smp.tile([D + 1, TQ], f32, name="ysb_b", tag="ysb",
                                     bufs=6)
                    nc.vector.tensor_copy(out=ysb_a[:], in_=yps_a[:])
                    nc.vector.tensor_copy(out=ysb_b[:], in_=yps_b[:])
                    # normalization rides the filler stream a few slots deep
                    # so the DVE reciprocal finishes before its broadcast
                    # matmul reaches the PE
                    steps = norm_steps(ha, qc, ysb_a) + norm_steps(hb, qc, ysb_b)
                    for i_, st in enumerate(steps):
                        fillers.insert(min(6 + i_, len(fillers)), ("norm", st))
            drain(len(fillers))

            # ---- c_proj (local half of the contraction) ----
            for it in range(NKT):
                for oc in range(2):
                    pso = psA.tile([128, TQ], f32, name="ps_o", tag="mm512")
                    for ic in range(4):
                        nc.tensor.matmul(
                            pso[:],
                            lhsT=yt[ic][:, it * 128 : (it + 1) * 128],
                            rhs=wp[ic][:, oc * TQ : (oc + 1) * TQ],
                            start=(ic == 0),
                            stop=(ic == 3),
                        )
                    ot = sp.tile([128, TQ], bf16, name="ot")
                    nc.vector.tensor_copy(out=ot[:], in_=pso[:])
                    nc.sync.dma_start(
                        out=out_d[it * 128 : (it + 1) * 128, oc * TQ : (oc + 1) * TQ],
                        in_=ot[:],
                    )

    nc.finalize()
    return nc


def _bf16(a):
    return np.ascontiguousarray(a, dtype=np.float32).astype(ml_dtypes.bfloat16)


def _core_inputs(x, w_attn, b_attn, w_proj, masks, core):
    b, g = divmod(core, 2)
    gs = slice(g * CL, (g + 1) * CL)
    wq, wk, wv_ = (w_attn[i * C : (i + 1) * C][gs] for i in range(3))
    bq, bk, bv_ = (b_attn[i * C : (i + 1) * C][gs] for i in range(3))

    wqkT = np.concatenate([wq, wk], 0).T            # [C, 2*CL]
    wvT = wv_.T                                     # [C, CL]
    wv_aug = np.zeros((C, VW), np.float32)
    bv_aug = np.zeros((1, VW), np.float32)
    for h in range(HL):
        wv_aug[:, h * 65 : h * 65 + 64] = wvT[:, h * 64 : (h + 1) * 64]
        bv_aug[0, h * 65 : h * 65 + 64] = bv_[h * 64 : (h + 1) * 64]
        bv_aug[0, h * 65 + 64] = 1.0                # softmax denominator column

    bqk_cat = np.concatenate([bq, bk])              # [2*CL]
    bqkc = np.stack(
        [bqk_cat[j * 128 : (j + 1) * 128] for j in range(8)], axis=1
    ).astype(np.float32)                            # [128, 8]

    return {
        "xt": _bf16(x[b].T).reshape(8, 128, T),
        "wqk": _bf16(wqkT).reshape(8, 128, 2 * CL),
        "wv": _bf16(wv_aug).reshape(8, 128, VW),
        "bqkc": bqkc,
        "bvb": np.ascontiguousarray(
            np.broadcast_to(bv_aug, (128, VW)), np.float32
        ),
        "wp": _bf16(w_proj[:, gs].T).reshape(4, 128, C),
        "mask": masks,
    }


def _make_masks():
    qq = np.arange(TQ)[None, :]
    kk = np.arange(TK)[:, None]
    m = np.stack([(qq >= kk + j * TK) for j in range(4)]).astype(np.float32)
    return m.astype(ml_dtypes.bfloat16)


def kernel(x, w_attn, b_attn, w_proj, b_proj):
    global _prog, last_results
    from concourse.bass_utils import run_bass_kernel_spmd

    if _prog is None:
        _prog = _build_program()

    x = np.asarray(x, np.float32)
    w_attn = np.asarray(w_attn, np.float32)
    b_attn = np.asarray(b_attn, np.float32)
    w_proj = np.asarray(w_proj, np.float32)
    b_proj = np.asarray(b_proj, np.float32)

    masks = _make_masks()
    in_maps = [
        _core_inputs(x, w_attn, b_attn, w_proj, masks, core) for core in range(8)
    ]
    kwargs = {}
    tmpdir = os.environ.get("BASS_TMPDIR")
    if tmpdir:
        os.makedirs(tmpdir, exist_ok=True)
        kwargs["tmpdir"] = tmpdir
    res = run_bass_kernel_spmd(_prog, in_maps, list(range(8)), **kwargs)
    last_results = res

    out = np.empty((B, T, C), np.float32)
    for b in range(B):
        out[b] = (
            np.asarray(res.results[2 * b]["out"], np.float32)
            + np.asarray(res.results[2 * b + 1]["out"], np.float32)
            + b_proj
        )
    return out



# revision 22
# speedup vs baseline: 1.1071x; 1.1071x over previous
"""Causal self-attention (B=4, T=2048, C=1024, H=16) on 8 NeuronCores.

Sharding: core = (batch b, head-group g): data-parallel over B=4, tensor-
parallel over heads (2 groups x 8 heads).  Each core computes QKV + attention
for its 8 heads and the matching half of the c_proj contraction; the host
sums the two partial c_proj outputs per batch and adds b_proj.

Device layout notes:
  - all matmul operands bf16 (PE runs fp32 at 1/4 rate), PSUM f32
  - x, weights are pre-transposed on the host so every matmul contraction
    sits on the partition dim; no on-device transposes anywhere
  - QKV biases enter as K=1 rank-1 matmuls against a ones row
  - S is computed transposed ([keys, queries]); exp(S/8) on ScalarE with no
    max-subtraction (logits bounded ~+-4 for this problem's scale)
  - causality at tile granularity: k-tiles above the diagonal are skipped,
    diagonal tiles multiplied by precomputed 0/1 masks after exp; diagonal
    tiles further restrict S/exp/PV to their valid column range
  - softmax denominator = ones column appended to each head's V; PV matmul
    emits [y.T | denom] per (head, q-chunk); normalization = batched DVE
    reciprocal + one-hot-selector broadcast matmul + elementwise multiply

Scheduling notes (the performance-critical part):
  - the attention inner loop is ScalarE(exp)-bound; the PE would idle ~35%
    and the HAM activity monitor then clock-throttles it to 1.2 GHz.  To
    keep the PE dense, later head-pairs' QKV projection matmuls (and the
    second half of V) are drip-fed one-per-iteration into the attention
    k-loop as "filler" work, and each head's normalization is deferred into
    the next head's filler stream (its reciprocal would otherwise stall the
    in-order PE queue >3.4us and re-trip the throttle)
  - S+exp run DEPTH=3 k-tiles ahead of the PV consumer (software pipeline)
  - emission order == dependency-tracking order, so ensure() force-emits a
    pair's projection fillers before that pair's attention reads them
"""

import os

import numpy as np
import ml_dtypes

B, T, C, H = 4, 2048, 1024, 16
D = 64          # head dim
HL = 8          # heads per core
CL = HL * D     # 512 local channels
TQ = 512        # query chunk (matmul moving dim)
TK = 128        # key tile (psum partition dim)
NQC = T // TQ   # 4 query chunks
NKT = T // TK   # 16 key tiles
VW = HL * (D + 1)  # 520: V with per-head ones column

_prog = None
last_results = None  # BassKernelResults of the most recent run (for test.py)


def _build_program():
    import concourse.mybir as mybir
    import concourse.tile as tile
    from concourse import bacc

    f32 = mybir.dt.float32
    bf16 = mybir.dt.bfloat16
    EXP = mybir.ActivationFunctionType.Exp

    nc = bacc.Bacc("TRN2", target_bir_lowering=False, debug=False)

    xt_d = nc.dram_tensor("xt", [8, 128, T], bf16, kind="ExternalInput")
    wqk_d = nc.dram_tensor("wqk", [8, 128, 2 * CL], bf16, kind="ExternalInput")
    wv_d = nc.dram_tensor("wv", [8, 128, VW], bf16, kind="ExternalInput")
    # per-partition bias columns: bqkc[:, dst*4+g] = bias for (q|k, pair g);
    # bvb = bv broadcast down 128 partitions (incl. the ones column) so the
    # bias adds ride the psum->sbuf copies instead of K=1 rank-1 matmuls
    bqkc_d = nc.dram_tensor("bqkc", [128, 8], f32, kind="ExternalInput")
    bvb_d = nc.dram_tensor("bvb", [128, VW], f32, kind="ExternalInput")
    wp_d = nc.dram_tensor("wp", [4, 128, C], bf16, kind="ExternalInput")
    mask_d = nc.dram_tensor("mask", [4, 128, TQ], bf16, kind="ExternalInput")
    out_d = nc.dram_tensor("out", [T, C], bf16, kind="ExternalOutput")

    with tile.TileContext(nc) as tc:
        with (
            tc.tile_pool(name="persist", bufs=1) as pp,
            tc.tile_pool(name="ptpool", bufs=6) as ptp,
            tc.tile_pool(name="stage", bufs=3) as sp,
            tc.tile_pool(name="small", bufs=3) as smp,
            tc.tile_pool(name="psA", bufs=2, space="PSUM") as psA,
            tc.tile_pool(name="psF", bufs=2, space="PSUM") as psF,
            tc.tile_pool(name="psY", bufs=2, space="PSUM") as psY,
        ):
            # ---- load everything ----
            xt = [pp.tile([128, T], bf16, name=f"xt{k}") for k in range(8)]
            wqk = [pp.tile([128, 2 * CL], bf16, name=f"wqk{k}") for k in range(8)]
            wv = [pp.tile([128, VW], bf16, name=f"wv{k}") for k in range(8)]
            wp = [pp.tile([128, C], bf16, name=f"wp{k}") for k in range(4)]
            maskt = [pp.tile([128, TQ], bf16, name=f"mask{j}") for j in range(4)]
            bqkc = pp.tile([128, 8], f32, name="bqkc")
            bvb = pp.tile([128, VW], f32, name="bvb")
            # ones row for the K=1 denominator-broadcast matmul
            ones64 = pp.tile([1, 64], bf16, name="ones64")

            # interleave wqk/xt so QKV matmul k-step j can start as soon as
            # its two tiles land (~2us) instead of after the full input load
            for k in range(8):
                nc.sync.dma_start(out=wqk[k][:], in_=wqk_d[k])
                nc.sync.dma_start(out=xt[k][:], in_=xt_d[k])
                if k == 0:
                    nc.sync.dma_start(out=bqkc[:], in_=bqkc_d[:])
                    nc.sync.dma_start(out=bvb[:], in_=bvb_d[:])
            for k in range(8):
                nc.sync.dma_start(out=wv[k][:], in_=wv_d[k])
            for j in range(4):
                nc.sync.dma_start(out=maskt[j][:], in_=mask_d[j])
            for k in range(4):
                nc.sync.dma_start(out=wp[k][:], in_=wp_d[k])
            nc.vector.memset(ones64[:], 1.0)

            # ---- QKV projection ----
            # QT/KT in [channel, t] layout; channel tile g = head pair g
            qt = [pp.tile([128, T], bf16, name=f"qt{i}") for i in range(4)]
            kt = [pp.tile([128, T], bf16, name=f"kt{i}") for i in range(4)]
            # V in natural [t, channel] layout with a ones column per head
            vsb = [pp.tile([128, VW], bf16, name=f"v{i}") for i in range(NKT)]
            yt = [pp.tile([128, T], bf16, name=f"yt{i}") for i in range(4)]

            def emit_qk_steps(g):
                """One head-pair's Q.T and K.T projection as a list of
                single-matmul closures (PE filler units)."""
                steps = []
                for dst, off, bi in ((qt, 0, 0), (kt, CL, 4)):
                    for j in range(NQC):
                        ph = {}

                        def step(k, ph=ph, dst=dst, off=off, j=j, g=g, bi=bi):
                            if k == 0:
                                ph["ps"] = psF.tile(
                                    [128, TQ], f32, name="ps_f", tag="fill"
                                )
                            nc.tensor.matmul(
                                ph["ps"][:],
                                lhsT=wqk[k][:, off + g * 128 : off + (g + 1) * 128],
                                rhs=xt[k][:, j * TQ : (j + 1) * TQ],
                                start=(k == 0),
                                stop=(k == 7),
                            )
                            if k == 7:
                                # bias add rides the psum->sbuf copy (DVE)
                                nc.vector.tensor_scalar_add(
                                    out=dst[g][:, j * TQ : (j + 1) * TQ],
                                    in0=ph["ps"][:],
                                    scalar1=bqkc[:, bi + g : bi + g + 1],
                                )

                        for k in range(8):
                            steps.append(lambda k=k, step=step: step(k))
                return steps

            def emit_v_steps(h2):
                """V projection for 4 heads as single-matmul closures."""
                w0 = h2 * (VW // 2)
                steps = []
                for it in range(NKT):
                    ph = {}

                    def step(k, ph=ph, it=it, w0=w0):
                        if k == 0:
                            ph["ps"] = psF.tile([128, TQ], f32, name="ps_v",
                                                tag="fill")
                        nc.tensor.matmul(
                            ph["ps"][:, : VW // 2],
                            lhsT=xt[k][:, it * 128 : (it + 1) * 128],
                            rhs=wv[k][:, w0 : w0 + VW // 2],
                            start=(k == 0),
                            stop=(k == 7),
                        )
                        if k == 7:
                            # bias (incl. the denominator ones column) rides
                            # the psum->sbuf copy via the broadcast tile
                            nc.vector.tensor_add(
                                out=vsb[it][:, w0 : w0 + VW // 2],
                                in0=ph["ps"][:, : VW // 2],
                                in1=bvb[:, w0 : w0 + VW // 2],
                            )

                    for k in range(8):
                        steps.append(lambda k=k, step=step: step(k))
                return steps

            # pre-phase (dense PE work, warms HAM): pair 0 QK + first V half
            # (heads 0-3); the second half drains via the filler stream well
            # before attn pair 2 (heads 4-7) consumes it
            for step in emit_qk_steps(0):
                step()
            for step in emit_v_steps(0):
                step()

            # ---- attention ----
            # the attention inner loop is ScalarE(exp)-bound; drip-feed the
            # NEXT pair's QKV matmuls into the PE queue as filler so the PE
            # stays dense (otherwise HAM clock-throttles it to 1.2 GHz)
            fillers = []  # list of (tag, closure)

            def drain(n):
                for _ in range(n):
                    if fillers:
                        fillers.pop(0)[1]()

            def ensure(tag):
                """Force-emit every queued step up to the last one of `tag`
                (emission order == dependency-tracking order, so a pair's
                projection steps MUST be emitted before its attention reads)."""
                while any(t == tag for t, _ in fillers):
                    fillers.pop(0)[1]()

            def norm_steps(p, qc, ysb_a, ysb_b):
                """Per-(pair, q-chunk) normalization as filler steps: copy each
                head's ysb denominator row down to partition 0 (custom DVE ops
                ignore the AP base partition on HW!), 1/den via approx_fast
                (~51 ULP, ~5x cheaper than exact), bf16 cast, then ONE shared
                2-bank psum tile takes both heads' K=1 ones-broadcast matmuls
                (ring-friendly: one psA allocation per q-chunk), and the two
                elementwise multiplies write the normalized yt halves."""
                rcs = []
                for ysb in (ysb_a, ysb_b):
                    den0 = smp.tile([1, TQ], f32, name="den0", tag="den0", bufs=4)
                    rec1f = smp.tile([1, TQ], f32, name="rec1f", tag="recf", bufs=4)
                    rec1b = smp.tile([1, TQ], bf16, name="rec1b", tag="recb", bufs=4)
                    rcs.append((ysb, den0, rec1f, rec1b))
                ph = {}

                def bc_mm(i):
                    if i == 0:
                        ph["bcp"] = psA.tile(
                            [128, 2 * TQ], f32, name="bcp", tag="pair"
                        )
                    nc.tensor.matmul(
                        ph["bcp"][0:64, i * TQ : (i + 1) * TQ],
                        lhsT=ones64[0:1, :],
                        rhs=rcs[i][3][0:1, :],
                        start=True,
                        stop=True,
                    )

                def mul(i, h_):
                    g2_, po_ = h_ // 2, 64 * (h_ % 2)
                    nc.vector.tensor_mul(
                        yt[g2_][po_ : po_ + 64, qc * TQ : (qc + 1) * TQ],
                        rcs[i][0][0:64, :],
                        ph["bcp"][0:64, i * TQ : (i + 1) * TQ],
                    )

                steps = []
                for i in range(2):
                    ysb, den0, rec1f, rec1b = rcs[i]
                    steps += [
                        lambda ysb=ysb, den0=den0: nc.vector.tensor_copy(
                            out=den0[:], in_=ysb[64:65, :]
                        ),
                        lambda den0=den0, rec1f=rec1f:
                            nc.vector.reciprocal_approx_fast(
                                out=rec1f[:], in_=den0[:]
                            ),
                        lambda rec1f=rec1f, rec1b=rec1b: nc.vector.tensor_copy(
                            out=rec1b[:], in_=rec1f[:]
                        ),
                    ]
                steps += [
                    lambda: bc_mm(0),
                    lambda: bc_mm(1),
                    lambda: mul(0, 2 * p),
                    lambda: mul(1, 2 * p + 1),
                ]
                return steps

            # head PAIRS: the two heads of a pair have their K/Q channel
            # halves at partitions 0:64 / 64:128 of the same kt/qt tile, so
            # their K=64 S matmuls go to PE row groups (0,0) / (64,0) and run
            # CONCURRENTLY in the array (row tiling) — the pair's two S tiles
            # cost ~one 512-column stream instead of two.
            DEPTH = 3
            for p in range(4):
                ha, hb = 2 * p, 2 * p + 1
                if p < 3:
                    fillers.extend((f"qk{p + 1}", s) for s in emit_qk_steps(p + 1))
                    if p == 0:
                        fillers.extend(("vh1", s) for s in emit_v_steps(1))
                ensure(f"qk{p}")
                if p >= 2:
                    ensure("vh1")
                for qc in range(NQC):
                    ktop = (qc + 1) * (TQ // TK)  # causal: k tiles 0..ktop-1
                    yps_a = psY.tile([D + 1, TQ], f32, name="yps_a", tag="y")
                    yps_b = psY.tile([D + 1, TQ], f32, name="yps_b", tag="y")
                    pts = {}

                    def s_stage(ktl, qc=qc, p=p, pts=pts):
                        j = ktl - qc * (TQ // TK)
                        # diagonal tiles have no valid columns before col0
                        col0 = j * TK if j >= 0 else 0
                        # one 2-bank psum tile holds BOTH heads' S: head a in
                        # [col0:TQ] (bank 0), head b in [TQ+col0:] (bank 1)
                        ps = psA.tile([128, 2 * TQ], f32, name="ps_s", tag="pair")
                        nc.tensor.matmul(
                            ps[:, col0:TQ],
                            lhsT=kt[p][0:64, ktl * TK : (ktl + 1) * TK],
                            rhs=qt[p][0:64, qc * TQ + col0 : (qc + 1) * TQ],
                            start=True,
                            stop=True,
                        )
                        nc.tensor.matmul(
                            ps[:, TQ + col0 :],
                            lhsT=kt[p][64:128, ktl * TK : (ktl + 1) * TK],
                            rhs=qt[p][64:128, qc * TQ + col0 : (qc + 1) * TQ],
                            start=True,
                            stop=True,
                        )
                        pt_t = ptp.tile([128, 2 * TQ], bf16, name="pt")
                        # P.T = exp(S.T/sqrt(D)); logits bounded, no max pass.
                        # col0==0: ONE 1024-wide exp covers both heads,
                        # halving the ~260ns/instr ACT overhead
                        if col0 == 0:
                            nc.scalar.activation(
                                pt_t[:, :], ps[:, :], EXP, scale=0.125
                            )
                        else:
                            nc.scalar.activation(
                                pt_t[:, col0:TQ], ps[:, col0:TQ], EXP, scale=0.125
                            )
                            nc.scalar.activation(
                                pt_t[:, TQ + col0 :], ps[:, TQ + col0 :], EXP,
                                scale=0.125,
                            )
                        if j >= 0:  # diagonal: zero the acausal corner
                            # on GPSIMD (idle): keeps the mask off the busy
                            # in-order DVE queue so PV never waits behind
                            # ysb/filler copies
                            nc.gpsimd.tensor_mul(
                                pt_t[:, col0:TQ], pt_t[:, col0:TQ],
                                maskt[j][:, col0:],
                            )
                            nc.gpsimd.tensor_mul(
                                pt_t[:, TQ + col0 :], pt_t[:, TQ + col0 :],
                                maskt[j][:, col0:],
                            )
                        pts[ktl] = (pt_t, col0)

                    def pv_stage(ktl, ha=ha, hb=hb, pts=pts, yps_a=yps_a,
                                 yps_b=yps_b, ktop=ktop):
                        pt_t, col0 = pts.pop(ktl)
                        nc.tensor.matmul(
                            yps_a[:, col0:],
                            lhsT=vsb[ktl][:, ha * 65 : (ha + 1) * 65],
                            rhs=pt_t[:, col0:TQ],
                            start=(ktl == 0),
                            stop=(ktl == ktop - 1),
                        )
                        nc.tensor.matmul(
                            yps_b[:, col0:],
                            lhsT=vsb[ktl][:, hb * 65 : (hb + 1) * 65],
                            rhs=pt_t[:, TQ + col0 :],
                            start=(ktl == 0),
                            stop=(ktl == ktop - 1),
                        )

                    for ktl in range(ktop):
                        s_stage(ktl)
                        drain(3 if len(fillers) >= 60 else 2)
                        if ktl >= DEPTH:
                            pv_stage(ktl - DEPTH)
                    for ktl in range(max(0, ktop - DEPTH), ktop):
                        pv_stage(ktl)
                    # rows 0..63 = unnormalized y.T, row 64 = denominator
                    ysb_a = smp.tile([D + 1, TQ], f32, name="ysb_a", tag="ysb",
                                     bufs=6)
                    ysb_b = smp.tile([D + 1, TQ], f32, name="ysb_b", tag="ysb",
                                     bufs=6)
                    nc.vector.tensor_copy(out=ysb_a[:], in_=yps_a[:])
                    nc.vector.tensor_copy(out=ysb_b[:], in_=yps_b[:])
                    # normalization rides the filler stream a few slots deep
                    # so the DVE reciprocal finishes before its broadcast
                    # matmul reaches the PE
                    steps = norm_steps(p, qc, ysb_a, ysb_b)
                    for i_, st in enumerate(steps):
                        fillers.insert(min(6 + i_, len(fillers)), ("norm", st))
            drain(len(fillers))

            # ---- c_proj (local half of the contraction) ----
            # one 2-bank psum tile per 128-token row block: both 512-wide
            # output chunks, one psum->sbuf copy, one 1024-wide DMA
            for it in range(NKT):
                pso = psA.tile([128, 2 * TQ], f32, name="ps_o", tag="pair")
                for oc in range(2):
                    for ic in range(4):
                        nc.tensor.matmul(
                            pso[:, oc * TQ : (oc + 1) * TQ],
                            lhsT=yt[ic][:, it * 128 : (it + 1) * 128],
                            rhs=wp[ic][:, oc * TQ : (oc + 1) * TQ],
                            start=(ic == 0),
                            stop=(ic == 3),
                        )
                ot = sp.tile([128, 2 * TQ], bf16, name="ot")
                nc.vector.tensor_copy(out=ot[:], in_=pso[:])
                nc.sync.dma_start(
                    out=out_d[it * 128 : (it + 1) * 128, :],
                    in_=ot[:],
                )

    nc.finalize()
    return nc


def _bf16(a):
    return np.ascontiguousarray(a, dtype=np.float32).astype(ml_dtypes.bfloat16)


def _core_inputs(x, w_attn, b_attn, w_proj, masks, core):
    b, g = divmod(core, 2)
    gs = slice(g * CL, (g + 1) * CL)
    wq, wk, wv_ = (w_attn[i * C : (i + 1) * C][gs] for i in range(3))
    bq, bk, bv_ = (b_attn[i * C : (i + 1) * C][gs] for i in range(3))

    wqkT = np.concatenate([wq, wk], 0).T            # [C, 2*CL]
    wvT = wv_.T                                     # [C, CL]
    wv_aug = np.zeros((C, VW), np.float32)
    bv_aug = np.zeros((1, VW), np.float32)
    for h in range(HL):
        wv_aug[:, h * 65 : h * 65 + 64] = wvT[:, h * 64 : (h + 1) * 64]
        bv_aug[0, h * 65 : h * 65 + 64] = bv_[h * 64 : (h + 1) * 64]
        bv_aug[0, h * 65 + 64] = 1.0                # softmax denominator column

    bqk_cat = np.concatenate([bq, bk])              # [2*CL]
    bqkc = np.stack(
        [bqk_cat[j * 128 : (j + 1) * 128] for j in range(8)], axis=1
    ).astype(np.float32)                            # [128, 8]

    return {
        "xt": _bf16(x[b].T).reshape(8, 128, T),
        "wqk": _bf16(wqkT).reshape(8, 128, 2 * CL),
        "wv": _bf16(wv_aug).reshape(8, 128, VW),
        "bqkc": bqkc,
        "bvb": np.ascontiguousarray(
            np.broadcast_to(bv_aug, (128, VW)), np.float32
        ),
        "wp": _bf16(w_proj[:, gs].T).reshape(4, 128, C),
        "mask": masks,
    }


def _make_masks():
    qq = np.arange(TQ)[None, :]
    kk = np.arange(TK)[:, None]
    m = np.stack([(qq >= kk + j * TK) for j in range(4)]).astype(np.float32)
    return m.astype(ml_dtypes.bfloat16)


def kernel(x, w_attn, b_attn, w_proj, b_proj):
    global _prog, last_results
    from concourse.bass_utils import run_bass_kernel_spmd

    if _prog is None:
        _prog = _build_program()

    x = np.asarray(x, np.float32)
    w_attn = np.asarray(w_attn, np.float32)
    b_attn = np.asarray(b_attn, np.float32)
    w_proj = np.asarray(w_proj, np.float32)
    b_proj = np.asarray(b_proj, np.float32)

    masks = _make_masks()
    in_maps = [
        _core_inputs(x, w_attn, b_attn, w_proj, masks, core) for core in range(8)
    ]
    kwargs = {}
    tmpdir = os.environ.get("BASS_TMPDIR")
    if tmpdir:
        os.makedirs(tmpdir, exist_ok=True)
        kwargs["tmpdir"] = tmpdir
    res = run_bass_kernel_spmd(_prog, in_maps, list(range(8)), **kwargs)
    last_results = res

    out = np.empty((B, T, C), np.float32)
    for b in range(B):
        out[b] = (
            np.asarray(res.results[2 * b]["out"], np.float32)
            + np.asarray(res.results[2 * b + 1]["out"], np.float32)
            + b_proj
        )
    return out



# revision 27
# speedup vs baseline: 1.1499x; 1.0387x over previous
"""Causal self-attention (B=4, T=2048, C=1024, H=16) on 8 NeuronCores.

Sharding: core = (batch b, head-group g): data-parallel over B=4, tensor-
parallel over heads (2 groups x 8 heads).  Each core computes QKV + attention
for its 8 heads and the matching half of the c_proj contraction; the host
sums the two partial c_proj outputs per batch and adds b_proj.

Device layout notes:
  - all matmul operands bf16 (PE runs fp32 at 1/4 rate), PSUM f32
  - x, weights are pre-transposed on the host so every matmul contraction
    sits on the partition dim; no on-device transposes anywhere
  - QKV biases enter as K=1 rank-1 matmuls against a ones row
  - S is computed transposed ([keys, queries]); exp(S/8) on ScalarE with no
    max-subtraction (logits bounded ~+-4 for this problem's scale)
  - causality at tile granularity: k-tiles above the diagonal are skipped,
    diagonal tiles multiplied by precomputed 0/1 masks after exp; diagonal
    tiles further restrict S/exp/PV to their valid column range
  - softmax denominator = ones column appended to each head's V; PV matmul
    emits [y.T | denom] per (head, q-chunk); normalization = batched DVE
    reciprocal + one-hot-selector broadcast matmul + elementwise multiply

Scheduling notes (the performance-critical part):
  - the attention inner loop is ScalarE(exp)-bound; the PE would idle ~35%
    and the HAM activity monitor then clock-throttles it to 1.2 GHz.  To
    keep the PE dense, later head-pairs' QKV projection matmuls (and the
    second half of V) are drip-fed one-per-iteration into the attention
    k-loop as "filler" work, and each head's normalization is deferred into
    the next head's filler stream (its reciprocal would otherwise stall the
    in-order PE queue >3.4us and re-trip the throttle)
  - S+exp run DEPTH=3 k-tiles ahead of the PV consumer (software pipeline)
  - emission order == dependency-tracking order, so ensure() force-emits a
    pair's projection fillers before that pair's attention reads them
"""

import os

import numpy as np
import ml_dtypes

B, T, C, H = 4, 2048, 1024, 16
D = 64          # head dim
HL = 8          # heads per core
CL = HL * D     # 512 local channels
TQ = 512        # query chunk (matmul moving dim)
TK = 128        # key tile (psum partition dim)
NQC = T // TQ   # 4 query chunks
NKT = T // TK   # 16 key tiles
VW = HL * (D + 1)  # 520: V with per-head ones column

_prog = None
last_results = None  # BassKernelResults of the most recent run (for test.py)


def _build_program():
    import concourse.mybir as mybir
    import concourse.tile as tile
    from concourse import bacc

    f32 = mybir.dt.float32
    bf16 = mybir.dt.bfloat16
    EXP = mybir.ActivationFunctionType.Exp

    nc = bacc.Bacc("TRN2", target_bir_lowering=False, debug=False)

    xt_d = nc.dram_tensor("xt", [8, 128, T], bf16, kind="ExternalInput")
    wqk_d = nc.dram_tensor("wqk", [8, 128, 2 * CL], bf16, kind="ExternalInput")
    wv_d = nc.dram_tensor("wv", [8, 128, VW], bf16, kind="ExternalInput")
    # per-partition bias columns: bqkc[:, dst*4+g] = bias for (q|k, pair g);
    # bvb = bv broadcast down 128 partitions (incl. the ones column) so the
    # bias adds ride the psum->sbuf copies instead of K=1 rank-1 matmuls
    bqkc_d = nc.dram_tensor("bqkc", [128, 8], f32, kind="ExternalInput")
    bvb_d = nc.dram_tensor("bvb", [128, VW], f32, kind="ExternalInput")
    wp_d = nc.dram_tensor("wp", [4, 128, C], bf16, kind="ExternalInput")
    mask_d = nc.dram_tensor("mask", [4, 128, TQ], bf16, kind="ExternalInput")
    out_d = nc.dram_tensor("out", [T, C], bf16, kind="ExternalOutput")

    with tile.TileContext(nc) as tc:
        with (
            tc.tile_pool(name="persist", bufs=1) as pp,
            tc.tile_pool(name="ptpool", bufs=6) as ptp,
            tc.tile_pool(name="stage", bufs=3) as sp,
            tc.tile_pool(name="small", bufs=3) as smp,
            tc.tile_pool(name="psA", bufs=2, space="PSUM") as psA,
            tc.tile_pool(name="psF", bufs=2, space="PSUM") as psF,
            tc.tile_pool(name="psY", bufs=2, space="PSUM") as psY,
        ):
            # ---- load everything ----
            xt = [pp.tile([128, T], bf16, name=f"xt{k}") for k in range(8)]
            wqk = [pp.tile([128, 2 * CL], bf16, name=f"wqk{k}") for k in range(8)]
            wv = [pp.tile([128, VW], bf16, name=f"wv{k}") for k in range(8)]
            wp = [pp.tile([128, C], bf16, name=f"wp{k}") for k in range(4)]
            maskt = [pp.tile([128, TQ], bf16, name=f"mask{j}") for j in range(4)]
            bqkc = pp.tile([128, 8], f32, name="bqkc")
            bvb = pp.tile([128, VW], f32, name="bvb")
            # ones row for the K=1 denominator-broadcast matmul
            ones64 = pp.tile([1, 64], bf16, name="ones64")

            # interleave wqk/xt so QKV matmul k-step j can start as soon as
            # its two tiles land (~2us) instead of after the full input load
            for k in range(8):
                nc.sync.dma_start(out=wqk[k][:], in_=wqk_d[k])
                nc.sync.dma_start(out=xt[k][:], in_=xt_d[k])
                if k == 0:
                    nc.sync.dma_start(out=bqkc[:], in_=bqkc_d[:])
                    nc.sync.dma_start(out=bvb[:], in_=bvb_d[:])
            for k in range(8):
                nc.sync.dma_start(out=wv[k][:], in_=wv_d[k])
            for j in range(4):
                nc.sync.dma_start(out=maskt[j][:], in_=mask_d[j])
            for k in range(4):
                nc.sync.dma_start(out=wp[k][:], in_=wp_d[k])
            nc.vector.memset(ones64[:], 1.0)

            # ---- QKV projection ----
            # QT/KT in [channel, t] layout; channel tile g = head pair g
            qt = [pp.tile([128, T], bf16, name=f"qt{i}") for i in range(4)]
            kt = [pp.tile([128, T], bf16, name=f"kt{i}") for i in range(4)]
            # V in natural [t, channel] layout with a ones column per head
            vsb = [pp.tile([128, VW], bf16, name=f"v{i}") for i in range(NKT)]
            yt = [pp.tile([128, T], bf16, name=f"yt{i}") for i in range(4)]
            # c_proj partials (pairs 0-2 of the contraction), staged in bf16:
            # computed as pair-3 attention filler so only the thin ic=3 pass
            # remains after the last head
            osb = [pp.tile([128, 2 * TQ], bf16, name=f"osb{i}") for i in range(NKT)]

            def emit_qk_steps(g):
                """One head-pair's Q.T and K.T projection as a list of
                single-matmul closures (PE filler units)."""
                steps = []
                for dst, off, bi in ((qt, 0, 0), (kt, CL, 4)):
                    for j in range(NQC):
                        ph = {}

                        def step(k, ph=ph, dst=dst, off=off, j=j, g=g, bi=bi):
                            if k == 0:
                                ph["ps"] = psF.tile(
                                    [128, TQ], f32, name="ps_f", tag="fill"
                                )
                            nc.tensor.matmul(
                                ph["ps"][:],
                                lhsT=wqk[k][:, off + g * 128 : off + (g + 1) * 128],
                                rhs=xt[k][:, j * TQ : (j + 1) * TQ],
                                start=(k == 0),
                                stop=(k == 7),
                            )
                            if k == 7:
                                # bias add rides the psum->sbuf copy (DVE)
                                nc.vector.tensor_scalar_add(
                                    out=dst[g][:, j * TQ : (j + 1) * TQ],
                                    in0=ph["ps"][:],
                                    scalar1=bqkc[:, bi + g : bi + g + 1],
                                )

                        for k in range(8):
                            steps.append(lambda k=k, step=step: step(k))
                return steps

            def emit_v_steps(h2):
                """V projection for 4 heads as single-matmul closures."""
                w0 = h2 * (VW // 2)
                steps = []
                for it in range(NKT):
                    ph = {}

                    def step(k, ph=ph, it=it, w0=w0):
                        if k == 0:
                            ph["ps"] = psF.tile([128, TQ], f32, name="ps_v",
                                                tag="fill")
                        nc.tensor.matmul(
                            ph["ps"][:, : VW // 2],
                            lhsT=xt[k][:, it * 128 : (it + 1) * 128],
                            rhs=wv[k][:, w0 : w0 + VW // 2],
                            start=(k == 0),
                            stop=(k == 7),
                        )
                        if k == 7:
                            # bias (incl. the denominator ones column) rides
                            # the psum->sbuf copy via the broadcast tile
                            nc.vector.tensor_add(
                                out=vsb[it][:, w0 : w0 + VW // 2],
                                in0=ph["ps"][:, : VW // 2],
                                in1=bvb[:, w0 : w0 + VW // 2],
                            )

                    for k in range(8):
                        steps.append(lambda k=k, step=step: step(k))
                return steps

            def emit_cproj3_steps():
                """c_proj contraction steps for head pairs 0-2 as filler units
                (3 matmuls + bf16 copy per output chunk).  Feeds pair-3's
                attention loop, which otherwise runs out of projection filler
                and goes ACT-bound."""
                steps = []
                for it in range(NKT):
                    for oc in range(2):
                        ph = {}

                        def step(ic, ph=ph, it=it, oc=oc):
                            if ic == 0:
                                ph["ps"] = psF.tile(
                                    [128, TQ], f32, name="ps_c3", tag="fill"
                                )
                            nc.tensor.matmul(
                                ph["ps"][:],
                                lhsT=yt[ic][:, it * 128 : (it + 1) * 128],
                                rhs=wp[ic][:, oc * TQ : (oc + 1) * TQ],
                                start=(ic == 0),
                                stop=(ic == 2),
                            )
                            if ic == 2:
                                nc.vector.tensor_copy(
                                    out=osb[it][:, oc * TQ : (oc + 1) * TQ],
                                    in_=ph["ps"][:],
                                )

                        for ic in range(3):
                            steps.append(lambda ic=ic, step=step: step(ic))
                return steps

            # pre-phase (dense PE work, warms HAM): pair 0 QK + first V half
            # (heads 0-3); the second half drains via the filler stream well
            # before attn pair 2 (heads 4-7) consumes it
            for step in emit_qk_steps(0):
                step()
            for step in emit_v_steps(0):
                step()

            # ---- attention ----
            # the attention inner loop is ScalarE(exp)-bound; drip-feed the
            # NEXT pair's QKV matmuls into the PE queue as filler so the PE
            # stays dense (otherwise HAM clock-throttles it to 1.2 GHz)
            fillers = []  # list of (tag, closure)

            def drain(n):
                for _ in range(n):
                    if fillers:
                        fillers.pop(0)[1]()

            def ensure(tag):
                """Force-emit every queued step up to the last one of `tag`
                (emission order == dependency-tracking order, so a pair's
                projection steps MUST be emitted before its attention reads)."""
                while any(t == tag for t, _ in fillers):
                    fillers.pop(0)[1]()

            def norm_steps(p, qc, ysb_a, ysb_b):
                """Per-(pair, q-chunk) normalization as filler steps: copy each
                head's ysb denominator row down to partition 0 (custom DVE ops
                ignore the AP base partition on HW!), 1/den via approx_fast
                (~51 ULP, ~5x cheaper than exact), bf16 cast, then ONE shared
                2-bank psum tile takes both heads' K=1 ones-broadcast matmuls
                (ring-friendly: one psA allocation per q-chunk), and the two
                elementwise multiplies write the normalized yt halves."""
                rcs = []
                for ysb in (ysb_a, ysb_b):
                    den0 = smp.tile([1, TQ], f32, name="den0", tag="den0", bufs=2)
                    rec1f = smp.tile([1, TQ], f32, name="rec1f", tag="recf", bufs=2)
                    rec1b = smp.tile([1, TQ], bf16, name="rec1b", tag="recb", bufs=2)
                    rcs.append((ysb, den0, rec1f, rec1b))
                ph = {}

                def bc_mm(i):
                    if i == 0:
                        ph["bcp"] = psA.tile(
                            [128, 2 * TQ], f32, name="bcp", tag="pair"
                        )
                    nc.tensor.matmul(
                        ph["bcp"][0:64, i * TQ : (i + 1) * TQ],
                        lhsT=ones64[0:1, :],
                        rhs=rcs[i][3][0:1, :],
                        start=True,
                        stop=True,
                    )

                def mul(i, h_):
                    g2_, po_ = h_ // 2, 64 * (h_ % 2)
                    nc.vector.tensor_mul(
                        yt[g2_][po_ : po_ + 64, qc * TQ : (qc + 1) * TQ],
                        rcs[i][0][0:64, :],
                        ph["bcp"][0:64, i * TQ : (i + 1) * TQ],
                    )

                steps = []
                for i in range(2):
                    ysb, den0, rec1f, rec1b = rcs[i]
                    steps += [
                        lambda ysb=ysb, den0=den0: nc.vector.tensor_copy(
                            out=den0[:], in_=ysb[64:65, :]
                        ),
                        lambda den0=den0, rec1f=rec1f:
                            nc.vector.reciprocal_approx_fast(
                                out=rec1f[:], in_=den0[:]
                            ),
                        lambda rec1f=rec1f, rec1b=rec1b: nc.vector.tensor_copy(
                            out=rec1b[:], in_=rec1f[:]
                        ),
                    ]
                steps += [
                    lambda: bc_mm(0),
                    lambda: bc_mm(1),
                    lambda: mul(0, 2 * p),
                    lambda: mul(1, 2 * p + 1),
                ]
                return steps

            # head PAIRS: the two heads of a pair have their K/Q channel
            # halves at partitions 0:64 / 64:128 of the same kt/qt tile, so
            # their K=64 S matmuls go to PE row groups (0,0) / (64,0) and run
            # CONCURRENTLY in the array (row tiling) — the pair's two S tiles
            # cost ~one 512-column stream instead of two.
            DEPTH = 3
            for p in range(4):
                ha, hb = 2 * p, 2 * p + 1
                if p < 3:
                    fillers.extend((f"qk{p + 1}", s) for s in emit_qk_steps(p + 1))
                    if p == 0:
                        fillers.extend(("vh1", s) for s in emit_v_steps(1))
                else:
                    fillers.extend(("cp3", s) for s in emit_cproj3_steps())
                ensure(f"qk{p}")
                if p >= 2:
                    ensure("vh1")
                for qc in range(NQC):
                    ktop = (qc + 1) * (TQ // TK)  # causal: k tiles 0..ktop-1
                    yps_a = psY.tile([D + 1, TQ], f32, name="yps_a", tag="y")
                    yps_b = psY.tile([D + 1, TQ], f32, name="yps_b", tag="y")
                    pts = {}

                    def s_stage(ktl, qc=qc, p=p, pts=pts):
                        j = ktl - qc * (TQ // TK)
                        # diagonal tiles have no valid columns before col0
                        col0 = j * TK if j >= 0 else 0
                        # one 2-bank psum tile holds BOTH heads' S: head a in
                        # [col0:TQ] (bank 0), head b in [TQ+col0:] (bank 1)
                        ps = psA.tile([128, 2 * TQ], f32, name="ps_s", tag="pair")
                        nc.tensor.matmul(
                            ps[:, col0:TQ],
                            lhsT=kt[p][0:64, ktl * TK : (ktl + 1) * TK],
                            rhs=qt[p][0:64, qc * TQ + col0 : (qc + 1) * TQ],
                            start=True,
                            stop=True,
                        )
                        nc.tensor.matmul(
                            ps[:, TQ + col0 :],
                            lhsT=kt[p][64:128, ktl * TK : (ktl + 1) * TK],
                            rhs=qt[p][64:128, qc * TQ + col0 : (qc + 1) * TQ],
                            start=True,
                            stop=True,
                        )
                        pt_t = ptp.tile([128, 2 * TQ], bf16, name="pt")
                        # P.T = exp(S.T/sqrt(D)); logits bounded, no max pass.
                        # col0==0: ONE 1024-wide exp covers both heads,
                        # halving the ~260ns/instr ACT overhead
                        if col0 == 0:
                            nc.scalar.activation(
                                pt_t[:, :], ps[:, :], EXP, scale=0.125
                            )
                        else:
                            nc.scalar.activation(
                                pt_t[:, col0:TQ], ps[:, col0:TQ], EXP, scale=0.125
                            )
                            nc.scalar.activation(
                                pt_t[:, TQ + col0 :], ps[:, TQ + col0 :], EXP,
                                scale=0.125,
                            )
                        if j >= 0:  # diagonal: zero the acausal corner
                            # on GPSIMD (idle): keeps the mask off the busy
                            # in-order DVE queue so PV never waits behind
                            # ysb/filler copies
                            nc.gpsimd.tensor_mul(
                                pt_t[:, col0:TQ], pt_t[:, col0:TQ],
                                maskt[j][:, col0:],
                            )
                            nc.gpsimd.tensor_mul(
                                pt_t[:, TQ + col0 :], pt_t[:, TQ + col0 :],
                                maskt[j][:, col0:],
                            )
                        pts[ktl] = (pt_t, col0)

                    def pv_stage(ktl, ha=ha, hb=hb, pts=pts, yps_a=yps_a,
                                 yps_b=yps_b, ktop=ktop):
                        pt_t, col0 = pts.pop(ktl)
                        nc.tensor.matmul(
                            yps_a[:, col0:],
                            lhsT=vsb[ktl][:, ha * 65 : (ha + 1) * 65],
                            rhs=pt_t[:, col0:TQ],
                            start=(ktl == 0),
                            stop=(ktl == ktop - 1),
                        )
                        nc.tensor.matmul(
                            yps_b[:, col0:],
                            lhsT=vsb[ktl][:, hb * 65 : (hb + 1) * 65],
                            rhs=pt_t[:, TQ + col0 :],
                            start=(ktl == 0),
                            stop=(ktl == ktop - 1),
                        )

                    for ktl in range(ktop):
                        s_stage(ktl)
                        drain(3 if len(fillers) >= 60 else 2)
                        if ktl >= DEPTH:
                            pv_stage(ktl - DEPTH)
                    for ktl in range(max(0, ktop - DEPTH), ktop):
                        pv_stage(ktl)
                    # rows 0..63 = unnormalized y.T, row 64 = denominator
                    ysb_a = smp.tile([D + 1, TQ], f32, name="ysb_a", tag="ysb",
                                     bufs=6)
                    ysb_b = smp.tile([D + 1, TQ], f32, name="ysb_b", tag="ysb",
                                     bufs=6)
                    nc.vector.tensor_copy(out=ysb_a[:], in_=yps_a[:])
                    nc.vector.tensor_copy(out=ysb_b[:], in_=yps_b[:])
                    # normalization rides the filler stream a few slots deep
                    # so the DVE reciprocal finishes before its broadcast
                    # matmul reaches the PE
                    steps = norm_steps(p, qc, ysb_a, ysb_b)
                    for i_, st in enumerate(steps):
                        fillers.insert(min(6 + i_, len(fillers)), ("norm", st))
            drain(len(fillers))

            # ---- c_proj tail: only the ic=3 (pair 3) contraction step plus
            # the staged bf16 partial; one 1024-wide copy-add and DMA per
            # 128-token row block
            for it in range(NKT):
                pso = psA.tile([128, 2 * TQ], f32, name="ps_o", tag="pair")
                for oc in range(2):
                    nc.tensor.matmul(
                        pso[:, oc * TQ : (oc + 1) * TQ],
                        lhsT=yt[3][:, it * 128 : (it + 1) * 128],
                        rhs=wp[3][:, oc * TQ : (oc + 1) * TQ],
                        start=True,
                        stop=True,
                    )
                ot = sp.tile([128, 2 * TQ], bf16, name="ot")
                nc.vector.tensor_add(out=ot[:], in0=pso[:], in1=osb[it][:])
                nc.sync.dma_start(
                    out=out_d[it * 128 : (it + 1) * 128, :],
                    in_=ot[:],
                )

    nc.finalize()
    return nc


def _bf16(a):
    return np.ascontiguousarray(a, dtype=np.float32).astype(ml_dtypes.bfloat16)


def _core_inputs(x, w_attn, b_attn, w_proj, masks, core):
    b, g = divmod(core, 2)
    gs = slice(g * CL, (g + 1) * CL)
    wq, wk, wv_ = (w_attn[i * C : (i + 1) * C][gs] for i in range(3))
    bq, bk, bv_ = (b_attn[i * C : (i + 1) * C][gs] for i in range(3))

    wqkT = np.concatenate([wq, wk], 0).T            # [C, 2*CL]
    wvT = wv_.T                                     # [C, CL]
    wv_aug = np.zeros((C, VW), np.float32)
    bv_aug = np.zeros((1, VW), np.float32)
    for h in range(HL):
        wv_aug[:, h * 65 : h * 65 + 64] = wvT[:, h * 64 : (h + 1) * 64]
        bv_aug[0, h * 65 : h * 65 + 64] = bv_[h * 64 : (h + 1) * 64]
        bv_aug[0, h * 65 + 64] = 1.0                # softmax denominator column

    bqk_cat = np.concatenate([bq, bk])              # [2*CL]
    bqkc = np.stack(
        [bqk_cat[j * 128 : (j + 1) * 128] for j in range(8)], axis=1
    ).astype(np.float32)                            # [128, 8]

    return {
        "xt": _bf16(x[b].T).reshape(8, 128, T),
        "wqk": _bf16(wqkT).reshape(8, 128, 2 * CL),
        "wv": _bf16(wv_aug).reshape(8, 128, VW),
        "bqkc": bqkc,
        "bvb": np.ascontiguousarray(
            np.broadcast_to(bv_aug, (128, VW)), np.float32
        ),
        "wp": _bf16(w_proj[:, gs].T).reshape(4, 128, C),
        "mask": masks,
    }


def _make_masks():
    qq = np.arange(TQ)[None, :]
    kk = np.arange(TK)[:, None]
    m = np.stack([(qq >= kk + j * TK) for j in range(4)]).astype(np.float32)
    return m.astype(ml_dtypes.bfloat16)


def kernel(x, w_attn, b_attn, w_proj, b_proj):
    global _prog, last_results
    from concourse.bass_utils import run_bass_kernel_spmd

    if _prog is None:
        _prog = _build_program()

    x = np.asarray(x, np.float32)
    w_attn = np.asarray(w_attn, np.float32)
    b_attn = np.asarray(b_attn, np.float32)
    w_proj = np.asarray(w_proj, np.float32)
    b_proj = np.asarray(b_proj, np.float32)

    masks = _make_masks()
    in_maps = [
        _core_inputs(x, w_attn, b_attn, w_proj, masks, core) for core in range(8)
    ]
    kwargs = {}
    tmpdir = os.environ.get("BASS_TMPDIR")
    if tmpdir:
        os.makedirs(tmpdir, exist_ok=True)
        kwargs["tmpdir"] = tmpdir
    res = run_bass_kernel_spmd(_prog, in_maps, list(range(8)), **kwargs)
    last_results = res

    out = np.empty((B, T, C), np.float32)
    for b in range(B):
        out[b] = (
            np.asarray(res.results[2 * b]["out"], np.float32)
            + np.asarray(res.results[2 * b + 1]["out"], np.float32)
            + b_proj
        )
    return out

